# revision 13
# baseline (speedup 1.0000x reference)
"""Multi-head attention (B=2, T=2048, D=1024, H=16, no causal mask) on 8 trn2
NeuronCores — head-parallel sharding with an all-gather exchange before o_proj.

Sharding: core c -> batch b = c//4, head-group g = c%4 (heads [4g, 4g+4), i.e.
local pairs p=0,1 = global pairs 2g+p).  Each core computes Q/K/V projections
and attention for its 4 heads over ALL 2048 queries of its batch (8.6 GFLOP
vs 15.1 for the old data-parallel scheme), then the cores of a batch exchange
attention outputs and each core runs o_proj for its 512-query slice
qs = [g*512, (g+1)*512).

Exchange: 8 small 4-core AllGathers (one per (pair, q-block)) fired as each
(pair, qb) block finishes normalize, so all but the last overlap attention.
AllToAll is unsupported for 4-core groups, so each AG delivers all 4 cores'
[128, 512] blocks and the consumer selects its own q-block with a per-core
one-hot mask input (SPMD-safe: rank enters via input data only) using fused
DVE multiply-adds.

Per-core pipeline (fp16 compute, fp32 PSUM):
  1. PE-transpose X[b] -> XT [128, kd*2048].
  2. QT/KT per pair: [128 (2 heads x 64), 2048]; V slots [128, kc*(4*65)]
     (65-wide: ones column makes PV also produce the softmax denominator).
  3. Attention pair-outer, qb-inner, kc innermost:
       logitsT via two row-tiled (tile_position (0,0)/(64,0)) K=64 matmuls
       that run CONCURRENTLY on the PE -> lg psum [128, 1024]
       exp on ScalarE (the only ACT-engine work in the kernel; ~138 us total,
       near-critical) -> PT f16
       PV accumulate [65, 512] psum over kc.
     normalize: 1/s via reciprocal_approx_fast, DMA partition-broadcast, DVE
     muls; head b partition-shifted via SBUF DMA.
  4. o_proj split: pair-0 slots accumulated into y_acc (SBUF f32) while
     pair-1 attention runs; pair-1 slots added at the tail.
Leftover projection work (V chunks, QT/KT pair 1) is emitted as PE filler
inside the attention loops (the attention inner loop alone is ACT-bound).
"""

import numpy as np

import concourse.bacc as bacc
import concourse.mybir as mybir
import concourse.tile as tile
from concourse.masks import make_identity

F32 = mybir.dt.float32
F16 = mybir.dt.float16

B, T, D, H = 2, 2048, 1024, 16
DH = D // H          # 64
P = 128
KD = D // P          # 8 contraction chunks over D
NT = T // P          # 16 key-token chunks
TQ = 512             # q-block size == o_proj q-slice
NQB = T // TQ        # 4 q blocks
NG = 4               # head-groups (cores per batch)
HPG = H // NG        # 4 heads per core
NPL = HPG // 2       # 2 local pairs
VW = DH + 1          # 65: V head slot width incl. ones column
NQC = TQ // P        # 4 128-row chunks per q-slice
N_CORES = 8
EXPF = mybir.ActivationFunctionType.Exp
MUL = mybir.AluOpType.mult
ADD = mybir.AluOpType.add

RG = [[0, 1, 2, 3], [4, 5, 6, 7]]


DEBUG_DUMPS = False
PRE_PROJ_P1 = True


def build_nc():
    nc = bacc.Bacc("TRN2", target_bir_lowering=False, debug=False,
                   num_devices=N_CORES)
    xb = nc.dram_tensor("xb", [T, D], F16, kind="ExternalInput").ap()
    wq = nc.dram_tensor("wq", [D, HPG * DH], F16, kind="ExternalInput").ap()
    wk = nc.dram_tensor("wk", [D, HPG * DH], F16, kind="ExternalInput").ap()
    wv = nc.dram_tensor("wv", [D, HPG * DH], F16, kind="ExternalInput").ap()
    wo = nc.dram_tensor("wo", [D, D], F16, kind="ExternalInput").ap()
    msk = nc.dram_tensor("msk", [P, NQB], F32, kind="ExternalInput").ap()
    y = nc.dram_tensor("y", [TQ, D], F32, kind="ExternalOutput").ap()
    if DEBUG_DUMPS:
        dbg_ot = nc.dram_tensor("dbg_ot", [P, NPL * T], F16,
                                kind="ExternalOutput").ap()
        dbg_sel = nc.dram_tensor("dbg_sel", [P, H // 2 * TQ], F16,
                                 kind="ExternalOutput").ap()
        dbg_qt = nc.dram_tensor("dbg_qt", [P, NPL * T], F16,
                                kind="ExternalOutput").ap()
        dbg_kt = nc.dram_tensor("dbg_kt", [P, NPL * T], F16,
                                kind="ExternalOutput").ap()

    with tile.TileContext(nc) as tc:
      with (
          tc.tile_pool(name="persist", bufs=1) as persist,
          tc.tile_pool(name="auxps", bufs=2, space="PSUM") as auxps,
          tc.tile_pool(name="xin", bufs=4) as xinp,
          tc.tile_pool(name="ptp", bufs=4) as ptp,
          tc.tile_pool(name="rbp", bufs=4) as rbp,
          tc.tile_pool(name="ginp", bufs=8) as ginp,
          tc.tile_pool(name="dram", bufs=1, space="DRAM") as dram,
      ):
        xt = persist.tile([P, KD * T], F16)          # 32 KB/part
        qt = persist.tile([P, NPL * T], F16)         # 8 KB
        kt = persist.tile([P, NPL * T], F16)         # 8 KB
        v_sb = persist.tile([P, NT * HPG * VW], F16)  # 8.1 KB
        ot = persist.tile([P, NPL * T], F16)         # 8 KB (own pairs out)
        ot_sel = persist.tile([P, H // 2 * TQ], F16)  # 8 KB (8 slots x 512)
        wo_sb = persist.tile([P, (H // 2) * D], F16)  # 16 KB [p, slot, 1024]
        y_acc = persist.tile([P, NQC * D], F32)      # 16 KB
        msk_sb = persist.tile([P, NQB], F32)
        ident = persist.tile([P, P], F16)
        make_identity(nc, ident)
        nc.sync.dma_start(msk_sb[:], msk)
        # wo rows grouped by global pair slot: wo_sb[p, s, :] = wo[s*128+p, :]
        nc.sync.dma_start(
            wo_sb.rearrange("p (s c) -> p s c", c=D),
            wo.rearrange("(s p) c -> p s c", p=P))
        # ones columns in every (tok-chunk, head) V slot
        onec = persist.tile([P, 1], F16)
        nc.vector.memset(onec[:], 1.0)
        nc.vector.tensor_copy(
            v_sb.rearrange("p (b c) -> p b c", c=VW)[:, :, DH:DH + 1],
            onec.unsqueeze(1).broadcast_to((P, NT * HPG, 1)))

        # weight slices viewed [p, kd, cols] for stationary use
        wq_sb = persist.tile([P, KD * HPG * DH], F16, name="wq_sb")  # 4 KB
        wk_sb = persist.tile([P, KD * HPG * DH], F16, name="wk_sb")
        wv_sb = persist.tile([P, KD * HPG * DH], F16, name="wv_sb")
        for w_sb, w in ((wq_sb, wq), (wk_sb, wk), (wv_sb, wv)):
            nc.sync.dma_start(
                w_sb.rearrange("p (kd c) -> p kd c", c=HPG * DH),
                w.rearrange("(kd p) c -> p kd c", p=P))

        ag_in = {}
        ag_out = {}
        for pl in range(NPL):
            for qb in range(NQB):
                ag_in[pl, qb] = dram.tile([P, TQ], F16,
                                          name=f"agin_{pl}_{qb}")
                ag_out[pl, qb] = dram.tile([NG, P, TQ], F16,
                                           name=f"agout_{pl}_{qb}")

        # ---------- helpers ------------------------------------------
        def proj_qk_block(w_sb, dst, pl, qb):
            # dst[:, pl*T + qb*TQ :] = W[:, pair pl]^T @ XT[:, qb block]
            pq = auxps.tile([P, TQ], F32, tag="aux")
            for kd in range(KD):
                nc.tensor.matmul(
                    pq[:],
                    w_sb[:, kd * HPG * DH + pl * P:
                         kd * HPG * DH + (pl + 1) * P],
                    xt[:, kd * T + qb * TQ: kd * T + (qb + 1) * TQ],
                    start=(kd == 0), stop=(kd == KD - 1))
            nc.vector.tensor_copy(
                dst[:, pl * T + qb * TQ: pl * T + (qb + 1) * TQ], pq[:])

        def proj_v_chunk(tci):
            # V rows for tokens [tci*128, ..): [128, 256] -> 65-wide slots
            pv = auxps.tile([P, TQ], F32, tag="aux")
            for kd in range(KD):
                nc.tensor.matmul(
                    pv[:, 0:HPG * DH],
                    xt[:, kd * T + tci * P: kd * T + (tci + 1) * P],
                    wv_sb[:, kd * HPG * DH:(kd + 1) * HPG * DH],
                    start=(kd == 0), stop=(kd == KD - 1))
            dst = v_sb[:, tci * (HPG * VW): (tci + 1) * (HPG * VW)]
            nc.vector.tensor_copy(
                dst.rearrange("p (h c) -> p h c", c=VW)[:, :, 0:DH],
                pv[:, 0:HPG * DH].rearrange("p (h c) -> p h c", c=DH))

        # select-accumulate AG output block into ot_sel slots
        def gather_select(pl, qb):
            for r in range(NG):
                gin = ginp.tile([P, TQ], F16, tag="gin")
                nc.sync.dma_start(gin[:], ag_out[pl, qb][r])
                slot = 2 * r + pl
                dst = ot_sel[:, slot * TQ:(slot + 1) * TQ]
                if qb == 0:
                    nc.vector.tensor_scalar_mul(
                        dst, gin[:], msk_sb[:, qb:qb + 1])
                else:
                    nc.vector.scalar_tensor_tensor(
                        dst, gin[:], msk_sb[:, qb:qb + 1], dst,
                        op0=MUL, op1=ADD)

        def oproj_half(pl):
            # accumulate this pair-half's 4 slots into y_acc / emit y
            for qc in range(NQC):
                for nh in range(2):
                    py = auxps.tile([P, TQ], F32, tag="aux")
                    for r in range(NG):
                        slot = 2 * r + pl
                        nc.tensor.matmul(
                            py[:],
                            ot_sel[:, slot * TQ + qc * P:
                                   slot * TQ + (qc + 1) * P],
                            wo_sb[:, slot * D + nh * TQ:
                                  slot * D + (nh + 1) * TQ],
                            start=(r == 0), stop=(r == NG - 1))
                    ya = y_acc[:, qc * D + nh * TQ: qc * D + (nh + 1) * TQ]
                    if pl == 0:
                        nc.vector.tensor_copy(ya, py[:])
                    else:
                        yo = rbp.tile([P, TQ], F32, tag="yout", bufs=3)
                        nc.vector.tensor_add(yo[:], py[:], ya)
                        nc.sync.dma_start(
                            y[qc * P:(qc + 1) * P, nh * TQ:(nh + 1) * TQ],
                            yo[:])

        # ---------- pre-region ---------------------------------------
        with tc.tile_pool(name="trps", bufs=3, space="PSUM") as trps:
            for tci in range(NT):
                xin = xinp.tile([P, D], F16, tag="xin")
                nc.sync.dma_start(xin[:], xb[tci * P:(tci + 1) * P, :])
                ps = trps.tile([P, KD * P], F16, tag="tr")
                for kd in range(KD):
                    nc.tensor.transpose(
                        ps[:, kd * P:(kd + 1) * P],
                        xin[:, kd * P:(kd + 1) * P], ident[:])
                nc.vector.tensor_copy(
                    xt.rearrange("p (k c) -> p k c", c=T)
                      [:, :, tci * P:(tci + 1) * P],
                    ps.rearrange("p (k c) -> p k c", c=P))

        for qb in range(NQB):
            proj_qk_block(wk_sb, kt, 0, qb)
        proj_qk_block(wq_sb, qt, 0, 0)
        for tci in range(2):
            proj_v_chunk(tci)

        # filler work emitted inside the attention loops, keyed by
        # (pair, qb, kc) slot.  Each item is a closure.
        filler = {}

        def add_filler(pl, qb, kc, fn):
            filler.setdefault((pl, qb, kc), []).append(fn)

        # V chunks tci=2..15 inside pair0 qb0: PV of kc reads v chunk kc,
        # so chunk tci must be emitted before PV tci — keep 2 ahead.
        for tci in range(2, NT):
            add_filler(0, 0, tci - 2, lambda t=tci: proj_v_chunk(t))
        # QT pair0 qb+1 emitted mid-way through qb
        for qb in range(NQB - 1):
            add_filler(0, qb, 10, lambda q=qb + 1:
                       proj_qk_block(wq_sb, qt, 0, q))
        # KT/QT pair1 during pair0 qb2/qb3
        for qb in range(NQB):
            add_filler(0, 2, 1 + 4 * qb, lambda q=qb:
                       proj_qk_block(wk_sb, kt, 1, q))
            add_filler(0, 3, 1 + 4 * qb, lambda q=qb:
                       proj_qk_block(wq_sb, qt, 1, q))
        if PRE_PROJ_P1:
            # debug/workaround: run pair-1 projections sequentially in the
            # pre-region instead of interleaved with attention.
            for key in [k for k in filler if k[0] == 0 and k[1] >= 2]:
                for fn in filler.pop(key):
                    fn()
        # pair0 gathers have landed by mid-pair1; fold their o_proj
        # half in as PE filler.
        add_filler(1, 1, 8, lambda: oproj_half(0))

        # ---------- attention ----------------------------------------
        with (
            tc.tile_pool(name="lgps", bufs=2, space="PSUM") as lgps,
            tc.tile_pool(name="pvps", bufs=2, space="PSUM") as pvps,
        ):
            for pl in range(NPL):
                for qb in range(NQB):
                    pva = pvps.tile([VW, TQ], F32, tag="pv")
                    pvb = pvps.tile([VW, TQ], F32, tag="pv")
                    for kc in range(NT):
                        for fn in filler.pop((pl, qb, kc), ()):
                            fn()
                        lg = lgps.tile([P, 2 * TQ], F32, tag="lg")
                        for hh in range(2):
                            nc.tensor.matmul(
                                lg[:, hh * TQ:(hh + 1) * TQ],
                                kt[hh * DH:(hh + 1) * DH,
                                   pl * T + kc * P: pl * T + (kc + 1) * P],
                                qt[hh * DH:(hh + 1) * DH,
                                   pl * T + qb * TQ: pl * T + (qb + 1) * TQ],
                                start=True, stop=True,
                                tile_position=(hh * DH, 0))
                        pt = ptp.tile([P, 2 * TQ], F16, tag="pt")
                        nc.scalar.activation(pt[:], lg[:], EXPF, scale=0.125)
                        for hh, pv_ in ((0, pva), (1, pvb)):
                            h = 2 * pl + hh
                            va = v_sb[:, kc * (HPG * VW) + h * VW:
                                      kc * (HPG * VW) + (h + 1) * VW]
                            nc.tensor.matmul(
                                pv_[:], va, pt[:, hh * TQ:(hh + 1) * TQ],
                                start=(kc == 0), stop=(kc == NT - 1))
                    # ---- normalize: ot[:, blk] = pv[0:64] / s ------------
                    srow = rbp.tile([1, 2 * TQ], F32, tag="srow")
                    nc.vector.tensor_copy(srow[0:1, 0:TQ],
                                          pva[DH:DH + 1, :])
                    nc.vector.tensor_copy(srow[0:1, TQ:2 * TQ],
                                          pvb[DH:DH + 1, :])
                    rc = rbp.tile([1, 2 * TQ], F32, tag="rc")
                    nc.vector.reciprocal_approx_fast(rc[:], srow[:])
                    rb = rbp.tile([P, TQ], F32, tag="rb")
                    rb2 = rbp.tile([P, TQ], F32, tag="rb2")
                    nc.sync.dma_start(
                        rb[0:DH, :],
                        rc[0:1, 0:TQ].unsqueeze(1)
                          .broadcast_to((1, DH, TQ)))
                    nc.sync.dma_start(
                        rb2[0:DH, :],
                        rc[0:1, TQ:2 * TQ].unsqueeze(1)
                          .broadcast_to((1, DH, TQ)))
                    dst = ot[:, pl * T + qb * TQ: pl * T + (qb + 1) * TQ]
                    nc.vector.tensor_mul(dst[0:DH, :], pva[0:DH, :],
                                         rb[0:DH, :])
                    # head b lands at partitions 64-127; DVE cannot shift
                    # partitions: normalize into staging then DMA-shift.
                    sh = rbp.tile([P, TQ], F16, tag="sh")
                    nc.vector.tensor_mul(sh[0:DH, :], pvb[0:DH, :],
                                         rb2[0:DH, :])
                    nc.sync.dma_start(dst[DH:P, :], sh[0:DH, :])
                    # ---- fire the exchange for this (pair, qb) block -----
                    nc.sync.dma_start(ag_in[pl, qb][:], dst)
                    nc.gpsimd.collective_compute(
                        "AllGather", mybir.AluOpType.bypass,
                        replica_groups=RG,
                        ins=[ag_in[pl, qb].opt()],
                        outs=[ag_out[pl, qb].opt()])
                    gather_select(pl, qb)

        # ---------- tail: pair-1 o_proj half + output ----------------
        oproj_half(1)
        assert not filler, f"unconsumed fillers: {list(filler)}"
        if DEBUG_DUMPS:
            nc.sync.dma_start(dbg_ot, ot[:])
            nc.sync.dma_start(dbg_sel, ot_sel[:])
            nc.sync.dma_start(dbg_qt, qt[:])
            nc.sync.dma_start(dbg_kt, kt[:])
    nc.compile()
    return nc


_NC_CACHE = None


def _get_nc():
    global _NC_CACHE
    if _NC_CACHE is None:
        _NC_CACHE = build_nc()
    return _NC_CACHE


def _shard_inputs(x, Wqkv, Wo):
    x16 = np.asarray(x, dtype=np.float32).astype(np.float16)
    w16 = np.asarray(Wqkv, dtype=np.float32).astype(np.float16)
    wo16 = np.ascontiguousarray(
        np.asarray(Wo, dtype=np.float32).astype(np.float16))
    in_maps = []
    for c in range(N_CORES):
        b, g = divmod(c, NG)
        cs = slice(g * HPG * DH, (g + 1) * HPG * DH)
        msk = np.zeros((P, NQB), dtype=np.float32)
        msk[:, g] = 1.0
        in_maps.append({
            "xb": np.ascontiguousarray(x16[b]),
            "wq": np.ascontiguousarray(w16[:, 0 * D:1 * D][:, cs]),
            "wk": np.ascontiguousarray(w16[:, 1 * D:2 * D][:, cs]),
            "wv": np.ascontiguousarray(w16[:, 2 * D:3 * D][:, cs]),
            "wo": wo16,
            "msk": msk,
        })
    return in_maps


def kernel(x, Wqkv, Wo):
    from concourse.bass_utils import run_bass_kernel_spmd

    nc = _get_nc()
    in_maps = _shard_inputs(x, Wqkv, Wo)
    res = run_bass_kernel_spmd(nc, in_maps, core_ids=list(range(N_CORES)))
    out = np.empty((B, T, D), dtype=np.float32)
    for c in range(N_CORES):
        b, g = divmod(c, NG)
        out[b, g * TQ:(g + 1) * TQ, :] = res.results[c]["y"]
    return out
